# revision 4
# baseline (speedup 1.0000x reference)
"""AxialAttention (axis=height) Trainium2 Bass kernel, v3.

Problem: x [B=2,T=4,C=256,H=128,W=128] f32. Lines run along H; N = B*T*W
independent sequences of length L=H=128 with C=256 channels, 8 heads x 32.
Sharding: one (b,t) pair per core (8 cores == B*T).

v3 engine-balance redesign (the baseline was Activation-bound at 93%):
  - scores PSUM tiles are [128, 1024] (2 banks) per (2 lines, 2 head-pairs):
    bank0 = heads (g, g+4) of both lines (one PE row band -> safe bank
    sharing), so ONE exp activation covers 1024 cols (amortizes the Act
    engine's SBUF-access init, ~185ns/op).
  - rel_bias handled multiplicatively post-exp: attnw = exp(SCALE*s) * eb.
    The multiply runs on DVE (2x bf16) for the A-tile and on GPSIMD (Pool,
    otherwise idle) for the B-tile of each pair.
  - softmax normalization via a single TensorTensor DIVIDE on the AV psum
    (kills the separate reciprocal).
  - qkv biases ride the PSUM->SBUF copies: Act activation bias or DVE
    tensor_scalar add (split to balance engines); softmax scale is folded
    into the exp's scale operand; bv is folded into bout on the host.
  - out-projection copies (+bout) on Act; x loads + out stores on the SP
    (sync) HWDGE queue.
PSUM budget (8 banks): scores 2x[128,1024]=4, V/Y pool 2, stageA/tr/proj
pool 2.
"""

import numpy as np
import ml_dtypes

import concourse.bacc as bacc
import concourse.bass as bass
import concourse.mybir as mybir
from concourse import tile
from concourse.bass import broadcast_tensor_aps
from concourse.bass_utils import run_bass_kernel_spmd

BF16 = ml_dtypes.bfloat16

B, T, C, H, W = 2, 4, 256, 128, 128
HEADS, DH = 8, 32
SCALE = DH ** (-0.5)
F = 3 * C  # 768
WBLK = 16
NBLK = W // WBLK  # 8
RBLK = H * WBLK  # 2048 block columns, (w, h) ordered
DT_B = mybir.dt.bfloat16
DT_F = mybir.dt.float32
AF = mybir.ActivationFunctionType
MUL = mybir.AluOpType.mult
DIV = mybir.AluOpType.divide
ADD = mybir.AluOpType.add

# fraction of stage-A copies on Act (rest DVE): index-based split
STAGEA_ACT = (0, 2)  # hq chunks handled by Act


def build_program():
    nc = bacc.Bacc("TRN2")

    x_bt = nc.dram_tensor("x_bt", [C, H, W], DT_B, kind="ExternalInput")
    # packed constants: [w1 768 | w2 768 | wo1 256 | wo2 256 | expbt 1024 | id 128]
    cb16 = nc.dram_tensor("cb16", [128, 3200], DT_B, kind="ExternalInput")
    # packed f32 biases: [bqk 4 | bout2 2]
    cf32 = nc.dram_tensor("cf32", [128, 6], DT_F, kind="ExternalInput")
    out_bt = nc.dram_tensor("out_bt", [C, H, W], DT_F, kind="ExternalOutput")

    with tile.TileContext(nc) as tc:
        with (
            tc.tile_pool(name="const", bufs=1) as cpool,
            tc.tile_pool(name="xt", bufs=1) as xt_pool,
            tc.tile_pool(name="qk", bufs=8) as qk_pool,
            tc.tile_pool(name="vp", bufs=1) as v_pool,
            tc.tile_pool(name="ex", bufs=4) as ex_pool,
            tc.tile_pool(name="at", bufs=4) as at_pool,
            tc.tile_pool(name="yn", bufs=4) as yn_pool,
            tc.tile_pool(name="yt", bufs=2) as yt_pool,
            tc.tile_pool(name="outp", bufs=2) as out_pool,
            tc.tile_pool(name="pssc", bufs=2, space="PSUM") as ps_sc,
            tc.tile_pool(name="psvy", bufs=2, space="PSUM") as ps_vy,
            tc.tile_pool(name="psms", bufs=2, space="PSUM") as ps_ms,
        ):
            # ---- constants (2 packed DMAs) ----
            cb = cpool.tile([128, 3200], DT_B, tag="cb16")
            nc.sync.dma_start(out=cb[:], in_=cb16[:])
            cf = cpool.tile([128, 6], DT_F, tag="cf32")
            nc.sync.dma_start(out=cf[:], in_=cf32[:])
            w1 = cb[:, 0:768]
            w2 = cb[:, 768:1536]
            wo1 = cb[:, 1536:1792]
            wo2 = cb[:, 1792:2048]
            eb_sb = cb[:, 2048:3072]
            id_sb = cb[:, 3072:3200]
            bqk_sb = cf[:, 0:4]
            bout_sb = cf[:, 4:6]

            wq = (w1, w2)

            # fence the (tiny) const loads so they never contribute sync
            # waits downstream; the big x load overlaps with compute
            tc.strict_bb_all_engine_barrier()

            # ---- resident X^T (host-cast bf16), h-quarters on the SP queue ----
            xt_all = xt_pool.tile([128, 2 * H * W], DT_B, tag="xt")
            xt_v = xt_all[:].rearrange("p (a f) -> p a f", a=2)
            for he in range(8):
                nc.sync.dma_start(
                    out=xt_v[:, :, he * 2048 : (he + 1) * 2048],
                    in_=x_bt[:, he * 16 : (he + 1) * 16, :].rearrange(
                        "(a c) h w -> c a (h w)", a=2
                    ),
                )
            # persistent per-line V tiles; ones columns written once
            v_tiles = []
            for j in range(WBLK):
                vt = v_pool.tile([128, HEADS * 33], DT_B, tag=f"vt{j}")
                nc.vector.memset(
                    vt[:].rearrange("p (a b) -> p a b", a=HEADS, b=33)[:, :, 32],
                    1.0,
                )
                v_tiles.append(vt)

            xt0 = xt_all[:, : H * W]
            xt1 = xt_all[:, H * W :]
            # (w, h)-major view of X^T columns: [c, w, h]
            xv0 = xt0.rearrange("p (h w) -> p w h", w=W)
            xv1 = xt1.rearrange("p (h w) -> p w h", w=W)
            xvs = (xv0, xv1)

            def stage_a(wb):
                """q/k feature-major tiles for this w-block; biases on copy."""
                w0 = wb * WBLK
                qk_tiles = []
                for ft in range(4):
                    qt = qk_pool.tile([128, RBLK], DT_B, tag="qkt")
                    qk_tiles.append(qt)
                    qtv = qt[:].rearrange("p (w h) -> p w h", w=WBLK)
                    for hq in range(4):
                        ps = ps_ms.tile([128, 512], DT_F, tag="ms")
                        for ct in range(2):
                            nc.tensor.matmul(
                                ps[:],
                                lhsT=wq[ct][:, ft * 128 : (ft + 1) * 128],
                                rhs=xvs[ct][
                                    :, w0 : w0 + WBLK, hq * 32 : (hq + 1) * 32
                                ],
                                start=(ct == 0),
                                stop=(ct == 1),
                            )
                        dst = qtv[:, :, hq * 32 : (hq + 1) * 32]
                        src = ps[:].rearrange("p (w h) -> p w h", w=WBLK)
                        if hq in STAGEA_ACT:
                            nc.scalar.activation(
                                dst, src, AF.Identity,
                                bias=bqk_sb[:, ft : ft + 1],
                            )
                        else:
                            nc.vector.tensor_scalar(
                                out=dst, in0=src,
                                scalar1=bqk_sb[:, ft : ft + 1],
                                scalar2=None, op0=ADD,
                            )
                return qk_tiles

            def v_stage(wb):
                """per-line V tiles [m, (8 heads, 33)], ones col persistent."""
                w0 = wb * WBLK
                for j in range(WBLK):
                    w = w0 + j
                    ps = ps_vy.tile([128, 256], DT_F, tag="vy")
                    for ct in range(2):
                        nc.tensor.matmul(
                            ps[:],
                            lhsT=xvs[ct][:, w, :],
                            rhs=wq[ct][:, 512:768],
                            start=(ct == 0),
                            stop=(ct == 1),
                        )
                    vt3 = v_tiles[j][:].rearrange(
                        "p (a b) -> p a b", a=HEADS, b=33
                    )
                    nc.vector.tensor_copy(
                        vt3[:, :, 0:32],
                        ps[:].rearrange("p (a b) -> p a b", a=HEADS, b=32),
                    )

            def scores_pair(qk_tiles, p):
                """two [128,1024] psum tiles for lines (2p, 2p+1).

                tile tt=0 ('A'): bank0 = heads (0,4) both lines (PE band 0),
                bank1 = heads (1,5) (band 32). tile 1 ('B'): heads (2,6) /
                (3,7) (bands 64/96). One band per bank -> concurrent-safe.
                """
                tiles = []
                for tt in range(2):
                    ps = ps_sc.tile([128, 1024], DT_F, tag="sc")
                    tiles.append(ps)
                    for g in range(2):
                        hb = tt * 2 + g  # band index 0..3
                        r0 = hb * 32
                        for jj in range(2):
                            jc = slice((2 * p + jj) * 128, (2 * p + jj + 1) * 128)
                            for hh in range(2):  # head = hb + 4*hh
                                nc.tensor.matmul(
                                    ps[:, g * 512 + jj * 256 + hh * 128 :
                                       g * 512 + jj * 256 + (hh + 1) * 128],
                                    lhsT=qk_tiles[2 + hh][r0 : r0 + 32, jc],
                                    rhs=qk_tiles[hh][r0 : r0 + 32, jc],
                                    start=True,
                                    stop=True,
                                    tile_position=(r0, 0),
                                )
                return tiles

            def exp_pair(sc_tiles):
                """exp(SCALE*s) -> bf16, one Act op per [128,1024] tile."""
                exps = []
                for tt in range(2):
                    ex = ex_pool.tile([128, 1024], DT_B, tag="ex")
                    nc.scalar.activation(ex[:], sc_tiles[tt][:], AF.Exp,
                                         scale=SCALE)
                    exps.append(ex)
                return exps

            def ebmul_pair(exps):
                """attnw = exps * exp(rel_bias^T); A-tile on DVE, B on Pool."""
                ats = []
                for tt in range(2):
                    at = at_pool.tile([128, 1024], DT_B, tag="at")
                    ats.append(at)
                    # tile cols = [g(2), line(2), (hd,l)(256)]; eb shared by
                    # both lines: eb cols [g(2), (hd,l)(256)] per tile
                    a3 = at[:].rearrange("p (g j c) -> p g j c", g=2, j=2)
                    e3 = exps[tt][:].rearrange("p (g j c) -> p g j c", g=2, j=2)
                    b3 = eb_sb[:, tt * 512 : (tt + 1) * 512].rearrange(
                        "p (g c) -> p g c", g=2
                    ).rearrange("p g (o c) -> p g o c", o=1)
                    i0, i1 = broadcast_tensor_aps(e3, b3)
                    eng = nc.vector if tt == 0 else nc.gpsimd
                    eng.tensor_tensor(a3, i0, i1, MUL)
                return ats

            # attnw col offset for (head h, line parity jj)
            def at_off(h, jj):
                g = (h % 4) % 2
                return g * 512 + jj * 256 + (h // 4) * 128

            def av_line(ats, j, jj, y_ps, half):
                """AV for heads of tile `half` (A: 0,4,1,5 / B: 2,6,3,7)."""
                for hb in (half * 2, half * 2 + 1):
                    for hh in range(2):
                        h = hb + 4 * hh
                        nc.tensor.matmul(
                            y_ps[:, h * 33 : h * 33 + 33],
                            lhsT=ats[half][:, at_off(h, jj) : at_off(h, jj) + 128],
                            rhs=v_tiles[j][:, h * 33 : h * 33 + 33],
                            start=True,
                            stop=True,
                        )

            def line_tail(j, y_ps, yt):
                """normalize by denominators (divide), transpose, store."""
                jc = slice(j * 128, (j + 1) * 128)
                y3 = y_ps[:].rearrange("p (a b) -> p a b", a=HEADS, b=33)
                yn = yn_pool.tile([128, C], DT_B, tag="yn")
                i0, i1 = broadcast_tensor_aps(y3[:, :, 0:32], y3[:, :, 32:33])
                nc.vector.tensor_tensor(
                    yn[:].rearrange("p (a b) -> p a b", a=HEADS, b=32),
                    i0, i1, DIV,
                )
                tr = ps_ms.tile([128, 256], DT_B, tag="ms")
                nc.tensor.transpose(tr[:, 0:128], yn[:, 0:128], id_sb[:])
                nc.tensor.transpose(tr[:, 128:256], yn[:, 128:256], id_sb[:])
                nc.vector.tensor_copy(
                    yt[:].rearrange("p (a l) -> p a l", a=2)[:, :, jc],
                    tr[:].rearrange("p (a l) -> p a l", a=2),
                )

            def proj_block(wb, yt, interleave):
                """output projection + bias copy + store; `interleave` emits
                stage-A/V matmul chunks of the next block between proj
                psum-tiles to keep the PE queue from head-blocking."""
                w0 = wb * WBLK
                it = iter(interleave)
                for ct in range(2):
                    ot = out_pool.tile([128, RBLK], DT_F, tag="ot")
                    otv = ot[:].rearrange("p (h w) -> p h w", w=WBLK)
                    for ch in range(RBLK // 512):
                        ps = ps_ms.tile([128, 512], DT_F, tag="ms")
                        nc.tensor.matmul(
                            ps[:],
                            lhsT=wo1[:, ct * 128 : (ct + 1) * 128],
                            rhs=yt[:, ch * 512 : (ch + 1) * 512],
                            start=True,
                            stop=False,
                        )
                        nc.tensor.matmul(
                            ps[:],
                            lhsT=wo2[:, ct * 128 : (ct + 1) * 128],
                            rhs=yt[:, RBLK + ch * 512 : RBLK + (ch + 1) * 512],
                            start=False,
                            stop=True,
                        )
                        # psum cols are (w 4, h 128); write reordered to (h, w)
                        nc.scalar.activation(
                            otv[:, :, ch * 4 : (ch + 1) * 4],
                            ps[:].rearrange("p (w h) -> p h w", w=4),
                            AF.Identity,
                            bias=bout_sb[:, ct : ct + 1],
                        )
                        for fn in next(it, ()):  # a few next-block emits
                            fn()
                    nc.sync.dma_start(
                        out=out_bt[ct * 128 : (ct + 1) * 128, :, w0 : w0 + WBLK],
                        in_=ot[:],
                    )
                for fns in it:
                    for fn in fns:
                        fn()

            # ================= main schedule =================
            # software pipeline: per pair p emit scores(p), exp(p), ebmul(p),
            # then AV/tails of pair p-1 (1-pair skew keeps the PE fed while
            # Act/DVE/Pool produce attnw).
            def attention_block(wb, qk_tiles, yt):
                prev = None  # (ats, j0)
                for p in range(NBLK):
                    sc = scores_pair(qk_tiles, p)
                    exps = exp_pair(sc)
                    ats = ebmul_pair(exps)
                    if prev is not None:
                        drain_pair(*prev, yt)
                    prev = (ats, 2 * p)
                drain_pair(*prev, yt)

            def drain_pair(ats, j0, yt):
                jl = j0 % WBLK
                y0 = ps_vy.tile([128, HEADS * 33], DT_F, tag="vy")
                y1 = ps_vy.tile([128, HEADS * 33], DT_F, tag="vy")
                # A-tile heads first (DVE finishes before Pool's B-tile)
                av_line(ats, jl, 0, y0, 0)
                av_line(ats, jl + 1, 1, y1, 0)
                av_line(ats, jl, 0, y0, 1)
                av_line(ats, jl + 1, 1, y1, 1)
                line_tail(jl, y0, yt)
                line_tail(jl + 1, y1, yt)

            pending_proj = None  # (wb, yt)
            for wb in range(NBLK):
                # interleave units for the previous block's proj: chunks of
                # next-block stage-A and V work, emitted between proj tiles
                if pending_proj is None:
                    qk_tiles = stage_a(wb)
                    v_stage(wb)
                else:
                    qk_holder = {}

                    def mk_all(_wb=wb, _h=qk_holder):
                        def go():
                            _h["t"] = stage_a(_wb)
                        return go

                    def mk_v(_wb=wb):
                        def go():
                            v_stage(_wb)
                        return go

                    units = [[mk_all()], [], [], [], [mk_v()], [], [], []]
                    proj_block(*pending_proj, interleave=units)
                    qk_tiles = qk_holder["t"]
                yt = yt_pool.tile([128, 2 * RBLK], DT_B, tag="yt")
                attention_block(wb, qk_tiles, yt)
                pending_proj = (wb, yt)
            proj_block(*pending_proj, interleave=[])

    nc.compile()
    return nc


_NC = None


def _get_nc():
    global _NC
    if _NC is None:
        _NC = build_program()
    return _NC


def _prep_small(rel_bias, Wqkv, bqkv, Wout, bout):
    # bf16 blob: [w1 768 | w2 768 | wo1 256 | wo2 256 | expbt 1024 | id 128]
    w12 = Wqkv.reshape(2, 128, F)
    wo12 = Wout.reshape(2, 128, C)
    expbt_a = np.exp(rel_bias.transpose(0, 2, 1))  # [hd, m, l]
    # head order (0,4),(1,5),(2,6),(3,7): pair (hd, hd+4) shares a PE row
    # band, so the pair's scores can share one PSUM bank safely
    expbt_a = expbt_a[[0, 4, 1, 5, 2, 6, 3, 7]]
    eb = expbt_a.transpose(1, 0, 2).reshape(128, HEADS * 128)  # [m, (hd, l)]
    cb16 = np.concatenate(
        [w12[0], w12[1], wo12[0], wo12[1], eb, np.eye(128, dtype=np.float32)],
        axis=1,
    ).astype(BF16)
    # raw biases (softmax scale folded into the exp activation's scale)
    bqk_a = np.stack(
        [bqkv[0:128], bqkv[128:256], bqkv[256:384], bqkv[384:512]],
        axis=1,
    )
    bout2_a = (bout + bqkv[512:] @ Wout).reshape(2, 128).T
    cf32 = np.concatenate([bqk_a, bout2_a], axis=1).astype(np.float32)
    return {"cb16": np.ascontiguousarray(cb16), "cf32": np.ascontiguousarray(cf32)}


def _run(x, rel_bias, Wqkv, bqkv, Wout, bout, **spmd_kwargs):
    x = np.asarray(x, dtype=np.float32)
    small = _prep_small(
        np.asarray(rel_bias, np.float32),
        np.asarray(Wqkv, np.float32),
        np.asarray(bqkv, np.float32),
        np.asarray(Wout, np.float32),
        np.asarray(bout, np.float32),
    )
    nc = _get_nc()
    core_ids = list(range(8))
    in_maps = []
    for i in core_ids:
        b, t = divmod(i, T)
        m = dict(small)
        m["x_bt"] = np.ascontiguousarray(x[b, t]).astype(BF16)
        in_maps.append(m)
    res = run_bass_kernel_spmd(nc, in_maps, core_ids, **spmd_kwargs)
    out = np.empty((B, T, C, H, W), np.float32)
    for i in core_ids:
        b, t = divmod(i, T)
        out[b, t] = res.results[i]["out_bt"]
    return out, res


def kernel(x, rel_bias, Wqkv, bqkv, Wout, bout):
    out, _ = _run(x, rel_bias, Wqkv, bqkv, Wout, bout)
    return out


# revision 10
# speedup vs baseline: 1.0472x; 1.0472x over previous
"""AxialAttention (axis=height) Trainium2 Bass kernel, v3.

Problem: x [B=2,T=4,C=256,H=128,W=128] f32. Lines run along H; N = B*T*W
independent sequences of length L=H=128 with C=256 channels, 8 heads x 32.
Sharding: one (b,t) pair per core (8 cores == B*T).

v3 engine-balance redesign (the baseline was Activation-bound at 93%):
  - scores PSUM tiles are [128, 1024] (2 banks) per (2 lines, 2 head-pairs):
    bank0 = heads (g, g+4) of both lines (one PE row band -> safe bank
    sharing), so ONE exp activation covers 1024 cols (amortizes the Act
    engine's SBUF-access init, ~185ns/op).
  - rel_bias handled multiplicatively post-exp: attnw = exp(SCALE*s) * eb.
    The multiply runs on DVE (2x bf16) for the A-tile and on GPSIMD (Pool,
    otherwise idle) for the B-tile of each pair.
  - softmax normalization via a single TensorTensor DIVIDE on the AV psum
    (kills the separate reciprocal).
  - qkv biases ride the PSUM->SBUF copies: Act activation bias or DVE
    tensor_scalar add (split to balance engines); softmax scale is folded
    into the exp's scale operand; bv is folded into bout on the host.
  - out-projection copies (+bout) on Act; x loads + out stores on the SP
    (sync) HWDGE queue.
PSUM budget (8 banks): scores 2x[128,1024]=4, V/Y pool 2, stageA/tr/proj
pool 2.
"""

import numpy as np
import ml_dtypes

import concourse.bacc as bacc
import concourse.bass as bass
import concourse.mybir as mybir
from concourse import tile
from concourse.bass import broadcast_tensor_aps
from concourse.bass_utils import run_bass_kernel_spmd

BF16 = ml_dtypes.bfloat16

B, T, C, H, W = 2, 4, 256, 128, 128
HEADS, DH = 8, 32
SCALE = DH ** (-0.5)
F = 3 * C  # 768
WBLK = 16
NBLK = W // WBLK  # 8
RBLK = H * WBLK  # 2048 block columns, (w, h) ordered
DT_B = mybir.dt.bfloat16
DT_F = mybir.dt.float32
AF = mybir.ActivationFunctionType
MUL = mybir.AluOpType.mult
DIV = mybir.AluOpType.divide
ADD = mybir.AluOpType.add

# stage-A copy split: hq chunk 1 goes to Act, rest to DVE (engine balance)
STAGEA_ACT = (1,)


def build_program():
    nc = bacc.Bacc("TRN2")

    x_bt = nc.dram_tensor("x_bt", [C, H, W], DT_B, kind="ExternalInput")
    # packed constants: [w1 768 | w2 768 | wo1 256 | wo2 256 | expbt 1024 | id 128]
    cb16 = nc.dram_tensor("cb16", [128, 3200], DT_B, kind="ExternalInput")
    # packed f32 biases: [bqk 4 | bout2 2]
    cf32 = nc.dram_tensor("cf32", [128, 6], DT_F, kind="ExternalInput")
    out_bt = nc.dram_tensor("out_bt", [C, H, W], DT_F, kind="ExternalOutput")

    with tile.TileContext(nc) as tc:
        with (
            tc.tile_pool(name="const", bufs=1) as cpool,
            tc.tile_pool(name="xt", bufs=1) as xt_pool,
            tc.tile_pool(name="qk", bufs=8) as qk_pool,
            tc.tile_pool(name="vp", bufs=1) as v_pool,
            tc.tile_pool(name="ex", bufs=4) as ex_pool,
            tc.tile_pool(name="at", bufs=4) as at_pool,
            tc.tile_pool(name="yn", bufs=4) as yn_pool,
            tc.tile_pool(name="yt", bufs=2) as yt_pool,
            tc.tile_pool(name="outp", bufs=2) as out_pool,
            tc.tile_pool(name="pssc", bufs=2, space="PSUM") as ps_sc,
            tc.tile_pool(name="psvy", bufs=2, space="PSUM") as ps_vy,
            tc.tile_pool(name="psms", bufs=2, space="PSUM") as ps_ms,
        ):
            # ---- constants (2 packed DMAs) ----
            cb = cpool.tile([128, 3200], DT_B, tag="cb16")
            nc.sync.dma_start(out=cb[:], in_=cb16[:])
            cf = cpool.tile([128, 6], DT_F, tag="cf32")
            nc.sync.dma_start(out=cf[:], in_=cf32[:])
            w1 = cb[:, 0:768]
            w2 = cb[:, 768:1536]
            wo1 = cb[:, 1536:1792]
            wo2 = cb[:, 1792:2048]
            eb_sb = cb[:, 2048:3072]
            id_sb = cb[:, 3072:3200]
            bqk_sb = cf[:, 0:4]
            bout_sb = cf[:, 4:6]

            wq = (w1, w2)

            # fence the (tiny) const loads so they never contribute sync
            # waits downstream; the big x load overlaps with compute
            tc.strict_bb_all_engine_barrier()

            # ---- resident X^T (host-cast bf16), h-quarters on the SP queue ----
            xt_all = xt_pool.tile([128, 2 * H * W], DT_B, tag="xt")
            xt_v = xt_all[:].rearrange("p (a f) -> p a f", a=2)
            for he in range(8):
                eng = nc.sync if he % 2 == 0 else nc.scalar
                eng.dma_start(
                    out=xt_v[:, :, he * 2048 : (he + 1) * 2048],
                    in_=x_bt[:, he * 16 : (he + 1) * 16, :].rearrange(
                        "(a c) h w -> c a (h w)", a=2
                    ),
                )
            # persistent per-line V tiles; ones columns written once
            v_tiles = []
            for j in range(WBLK):
                vt = v_pool.tile([128, HEADS * 33], DT_B, tag=f"vt{j}")
                nc.vector.memset(
                    vt[:].rearrange("p (a b) -> p a b", a=HEADS, b=33)[:, :, 32],
                    1.0,
                )
                v_tiles.append(vt)

            xt0 = xt_all[:, : H * W]
            xt1 = xt_all[:, H * W :]
            # (w, h)-major view of X^T columns: [c, w, h]
            xv0 = xt0.rearrange("p (h w) -> p w h", w=W)
            xv1 = xt1.rearrange("p (h w) -> p w h", w=W)
            xvs = (xv0, xv1)

            def stage_a(wb, hq_major=False):
                """q/k feature-major tiles for this w-block; biases on copy.

                hq_major=True (block 0) consumes the x h-quarters in DMA
                arrival order so compute starts before the full x load.
                """
                w0 = wb * WBLK
                qk_tiles = []
                for _ft in range(4):
                    qt = qk_pool.tile([128, RBLK], DT_B, tag="qkt")
                    qk_tiles.append(qt)
                order = (
                    [(ft, hq) for hq in range(4) for ft in range(4)]
                    if hq_major
                    else [(ft, hq) for ft in range(4) for hq in range(4)]
                )
                for ft, hq in order:
                    qtv = qk_tiles[ft][:].rearrange("p (w h) -> p w h", w=WBLK)
                    ps = ps_ms.tile([128, 512], DT_F, tag="ms")
                    for ct in range(2):
                        nc.tensor.matmul(
                            ps[:],
                            lhsT=wq[ct][:, ft * 128 : (ft + 1) * 128],
                            rhs=xvs[ct][
                                :, w0 : w0 + WBLK, hq * 32 : (hq + 1) * 32
                            ],
                            start=(ct == 0),
                            stop=(ct == 1),
                        )
                    dst = qtv[:, :, hq * 32 : (hq + 1) * 32]
                    src = ps[:].rearrange("p (w h) -> p w h", w=WBLK)
                    if hq in STAGEA_ACT:
                        nc.scalar.activation(
                            dst, src, AF.Identity,
                            bias=bqk_sb[:, ft : ft + 1],
                        )
                    else:
                        nc.vector.tensor_scalar(
                            out=dst, in0=src,
                            scalar1=bqk_sb[:, ft : ft + 1],
                            scalar2=None, op0=ADD,
                        )
                return qk_tiles

            def v_stage(wb):
                """per-line V tiles [m, (8 heads, 33)], ones col persistent."""
                w0 = wb * WBLK
                for j in range(WBLK):
                    w = w0 + j
                    ps = ps_vy.tile([128, 256], DT_F, tag="vy")
                    for ct in range(2):
                        nc.tensor.matmul(
                            ps[:],
                            lhsT=xvs[ct][:, w, :],
                            rhs=wq[ct][:, 512:768],
                            start=(ct == 0),
                            stop=(ct == 1),
                        )
                    vt3 = v_tiles[j][:].rearrange(
                        "p (a b) -> p a b", a=HEADS, b=33
                    )
                    nc.vector.tensor_copy(
                        vt3[:, :, 0:32],
                        ps[:].rearrange("p (a b) -> p a b", a=HEADS, b=32),
                    )

            def scores_pair(qk_tiles, p):
                """two [128,1024] psum tiles for lines (2p, 2p+1).

                tile tt=0 ('A'): bank0 = heads (0,4) both lines (PE band 0),
                bank1 = heads (1,5) (band 32). tile 1 ('B'): heads (2,6) /
                (3,7) (bands 64/96). One band per bank -> concurrent-safe.
                """
                tiles = []
                for tt in range(2):
                    ps = ps_sc.tile([128, 1024], DT_F, tag="sc")
                    tiles.append(ps)
                    for g in range(2):
                        hb = tt * 2 + g  # band index 0..3
                        r0 = hb * 32
                        for jj in range(2):
                            jc = slice((2 * p + jj) * 128, (2 * p + jj + 1) * 128)
                            for hh in range(2):  # head = hb + 4*hh
                                nc.tensor.matmul(
                                    ps[:, g * 512 + jj * 256 + hh * 128 :
                                       g * 512 + jj * 256 + (hh + 1) * 128],
                                    lhsT=qk_tiles[2 + hh][r0 : r0 + 32, jc],
                                    rhs=qk_tiles[hh][r0 : r0 + 32, jc],
                                    start=True,
                                    stop=True,
                                    tile_position=(r0, 0),
                                )
                return tiles

            def exp_pair(sc_tiles):
                """exp(SCALE*s) -> bf16, one Act op per [128,1024] tile."""
                exps = []
                for tt in range(2):
                    ex = ex_pool.tile([128, 1024], DT_B, tag="ex")
                    nc.scalar.activation(ex[:], sc_tiles[tt][:], AF.Exp,
                                         scale=SCALE)
                    exps.append(ex)
                return exps

            def ebmul_pair(exps):
                """attnw = exps * exp(rel_bias^T); A-tile on DVE, B on Pool."""
                ats = []
                for tt in range(2):
                    at = at_pool.tile([128, 1024], DT_B, tag="at")
                    ats.append(at)
                    # tile cols = [g(2), line(2), (hd,l)(256)]; eb shared by
                    # both lines: eb cols [g(2), (hd,l)(256)] per tile
                    a3 = at[:].rearrange("p (g j c) -> p g j c", g=2, j=2)
                    e3 = exps[tt][:].rearrange("p (g j c) -> p g j c", g=2, j=2)
                    b3 = eb_sb[:, tt * 512 : (tt + 1) * 512].rearrange(
                        "p (g c) -> p g c", g=2
                    ).rearrange("p g (o c) -> p g o c", o=1)
                    i0, i1 = broadcast_tensor_aps(e3, b3)
                    nc.gpsimd.tensor_tensor(a3, i0, i1, MUL)
                return ats

            # attnw col offset for (head h, line parity jj)
            def at_off(h, jj):
                g = (h % 4) % 2
                return g * 512 + jj * 256 + (h // 4) * 128

            def av_line(ats, j, jj, y_ps, half):
                """AV for heads of tile `half` (A: 0,4,1,5 / B: 2,6,3,7)."""
                for hb in (half * 2, half * 2 + 1):
                    for hh in range(2):
                        h = hb + 4 * hh
                        nc.tensor.matmul(
                            y_ps[:, h * 33 : h * 33 + 33],
                            lhsT=ats[half][:, at_off(h, jj) : at_off(h, jj) + 128],
                            rhs=v_tiles[j][:, h * 33 : h * 33 + 33],
                            start=True,
                            stop=True,
                        )

            def line_tail(j, y_ps, yt):
                """normalize by denominators (divide), transpose, store."""
                jc = slice(j * 128, (j + 1) * 128)
                y3 = y_ps[:].rearrange("p (a b) -> p a b", a=HEADS, b=33)
                yn = yn_pool.tile([128, C], DT_B, tag="yn")
                i0, i1 = broadcast_tensor_aps(y3[:, :, 0:32], y3[:, :, 32:33])
                nc.vector.tensor_tensor(
                    yn[:].rearrange("p (a b) -> p a b", a=HEADS, b=32),
                    i0, i1, DIV,
                )
                tr = ps_ms.tile([128, 256], DT_B, tag="ms")
                nc.tensor.transpose(tr[:, 0:128], yn[:, 0:128], id_sb[:])
                nc.tensor.transpose(tr[:, 128:256], yn[:, 128:256], id_sb[:])
                nc.vector.tensor_copy(
                    yt[:].rearrange("p (a l) -> p a l", a=2)[:, :, jc],
                    tr[:].rearrange("p (a l) -> p a l", a=2),
                )

            def proj_block(wb, yt, interleave):
                """output projection + bias copy + store; `interleave` emits
                stage-A/V matmul chunks of the next block between proj
                psum-tiles to keep the PE queue from head-blocking."""
                w0 = wb * WBLK
                it = iter(interleave)
                for ct in range(2):
                    ot = out_pool.tile([128, RBLK], DT_F, tag="ot")
                    otv = ot[:].rearrange("p (h w) -> p h w", w=WBLK)
                    for ch in range(RBLK // 512):
                        ps = ps_ms.tile([128, 512], DT_F, tag="ms")
                        nc.tensor.matmul(
                            ps[:],
                            lhsT=wo1[:, ct * 128 : (ct + 1) * 128],
                            rhs=yt[:, ch * 512 : (ch + 1) * 512],
                            start=True,
                            stop=False,
                        )
                        nc.tensor.matmul(
                            ps[:],
                            lhsT=wo2[:, ct * 128 : (ct + 1) * 128],
                            rhs=yt[:, RBLK + ch * 512 : RBLK + (ch + 1) * 512],
                            start=False,
                            stop=True,
                        )
                        # psum cols are (w 4, h 128); write reordered to (h, w)
                        nc.scalar.activation(
                            otv[:, :, ch * 4 : (ch + 1) * 4],
                            ps[:].rearrange("p (w h) -> p h w", w=4),
                            AF.Identity,
                            bias=bout_sb[:, ct : ct + 1],
                        )
                        for fn in next(it, ()):  # a few next-block emits
                            fn()
                    nc.sync.dma_start(
                        out=out_bt[ct * 128 : (ct + 1) * 128, :, w0 : w0 + WBLK],
                        in_=ot[:],
                    )
                for fns in it:
                    for fn in fns:
                        fn()

            # ================= main schedule =================
            # software pipeline: per pair p emit scores(p), exp(p), ebmul(p),
            # then AV/tails of pair p-1 (1-pair skew keeps the PE fed while
            # Act/DVE/Pool produce attnw).
            def attention_block(wb, qk_tiles, yt):
                prev = None  # (ats, j0)
                for p in range(NBLK):
                    sc = scores_pair(qk_tiles, p)
                    exps = exp_pair(sc)
                    ats = ebmul_pair(exps)
                    if prev is not None:
                        drain_pair(*prev, yt)
                    prev = (ats, 2 * p)
                drain_pair(*prev, yt)

            def drain_pair(ats, j0, yt):
                jl = j0 % WBLK
                y0 = ps_vy.tile([128, HEADS * 33], DT_F, tag="vy")
                y1 = ps_vy.tile([128, HEADS * 33], DT_F, tag="vy")
                # A-tile heads first (DVE finishes before Pool's B-tile)
                av_line(ats, jl, 0, y0, 0)
                av_line(ats, jl + 1, 1, y1, 0)
                av_line(ats, jl, 0, y0, 1)
                av_line(ats, jl + 1, 1, y1, 1)
                line_tail(jl, y0, yt)
                line_tail(jl + 1, y1, yt)

            pending_proj = None  # (wb, yt)
            for wb in range(NBLK):
                # interleave units for the previous block's proj: chunks of
                # next-block stage-A and V work, emitted between proj tiles
                if pending_proj is None:
                    qk_tiles = stage_a(wb, hq_major=True)
                    v_stage(wb)
                else:
                    qk_holder = {}

                    def mk_all(_wb=wb, _h=qk_holder):
                        def go():
                            _h["t"] = stage_a(_wb)
                        return go

                    def mk_v(_wb=wb):
                        def go():
                            v_stage(_wb)
                        return go

                    units = [[mk_all()], [], [], [], [mk_v()], [], [], []]
                    proj_block(*pending_proj, interleave=units)
                    qk_tiles = qk_holder["t"]
                yt = yt_pool.tile([128, 2 * RBLK], DT_B, tag="yt")
                attention_block(wb, qk_tiles, yt)
                pending_proj = (wb, yt)
            proj_block(*pending_proj, interleave=[])

    nc.compile()
    return nc


_NC = None


def _get_nc():
    global _NC
    if _NC is None:
        _NC = build_program()
    return _NC


def _prep_small(rel_bias, Wqkv, bqkv, Wout, bout):
    # bf16 blob: [w1 768 | w2 768 | wo1 256 | wo2 256 | expbt 1024 | id 128]
    w12 = Wqkv.reshape(2, 128, F)
    wo12 = Wout.reshape(2, 128, C)
    expbt_a = np.exp(rel_bias.transpose(0, 2, 1))  # [hd, m, l]
    # head order (0,4),(1,5),(2,6),(3,7): pair (hd, hd+4) shares a PE row
    # band, so the pair's scores can share one PSUM bank safely
    expbt_a = expbt_a[[0, 4, 1, 5, 2, 6, 3, 7]]
    eb = expbt_a.transpose(1, 0, 2).reshape(128, HEADS * 128)  # [m, (hd, l)]
    cb16 = np.concatenate(
        [w12[0], w12[1], wo12[0], wo12[1], eb, np.eye(128, dtype=np.float32)],
        axis=1,
    ).astype(BF16)
    # raw biases (softmax scale folded into the exp activation's scale)
    bqk_a = np.stack(
        [bqkv[0:128], bqkv[128:256], bqkv[256:384], bqkv[384:512]],
        axis=1,
    )
    bout2_a = (bout + bqkv[512:] @ Wout).reshape(2, 128).T
    cf32 = np.concatenate([bqk_a, bout2_a], axis=1).astype(np.float32)
    return {"cb16": np.ascontiguousarray(cb16), "cf32": np.ascontiguousarray(cf32)}


def _run(x, rel_bias, Wqkv, bqkv, Wout, bout, **spmd_kwargs):
    x = np.asarray(x, dtype=np.float32)
    small = _prep_small(
        np.asarray(rel_bias, np.float32),
        np.asarray(Wqkv, np.float32),
        np.asarray(bqkv, np.float32),
        np.asarray(Wout, np.float32),
        np.asarray(bout, np.float32),
    )
    nc = _get_nc()
    core_ids = list(range(8))
    in_maps = []
    for i in core_ids:
        b, t = divmod(i, T)
        m = dict(small)
        m["x_bt"] = np.ascontiguousarray(x[b, t]).astype(BF16)
        in_maps.append(m)
    res = run_bass_kernel_spmd(nc, in_maps, core_ids, **spmd_kwargs)
    out = np.empty((B, T, C, H, W), np.float32)
    for i in core_ids:
        b, t = divmod(i, T)
        out[b, t] = res.results[i]["out_bt"]
    return out, res


def kernel(x, rel_bias, Wqkv, bqkv, Wout, bout):
    out, _ = _run(x, rel_bias, Wqkv, bqkv, Wout, bout)
    return out


# revision 12
# speedup vs baseline: 1.2315x; 1.1760x over previous
"""AxialAttention (axis=height) Trainium2 Bass kernel, v4.

Problem: x [B=2,T=4,C=256,H=128,W=128] f32. Lines run along H; N = B*T*W
independent sequences of length L=H=128 with C=256 channels, 8 heads x 32.
Sharding: one (b,t) pair per core (8 cores == B*T).

Engine-balance + pipeline design (baseline was Activation-bound at 93%):
  - x is host-transposed to [C, W, H] so each w-block's stage-A/V work
    depends on exactly one of the 8 streamed x DMA chunks (fast start).
  - scores PSUM tiles are [128, 1024] (2 banks) per (2 lines, 2 head-pair
    groups): bank0 holds heads (g, g+4) of both lines -> one PE row band
    per bank (concurrent-matmul-safe), and ONE exp activation covers 1024
    cols, amortizing the Act engine's access-init (~185ns/op).
  - rel_bias applied multiplicatively post-exp on the otherwise-idle
    GPSIMD/Pool engine (SBUF-only engine; it cannot touch PSUM).
  - softmax normalization is a single TensorTensor DIVIDE by the
    denominator column (ones-column trick in the AV matmul).
  - qkv biases ride the PSUM->SBUF copies (Act activation bias / DVE
    tensor_scalar add); softmax scale is folded into exp's scale; bv is
    folded into bout on the host.
  - flat 64-slot software pipeline (8 blocks x 8 line-pairs): slot s does
    scores/exp/ebmul(s), AV+normalize(s-1), transpose+store(s-2), plus
    phase-scheduled filler work (prev block's out-projection, next block's
    stage-A/V) to keep all queues deep without PSUM over-subscription.
PSUM (8 banks): scores+proj pool 2x[128,1024]=4, V/Y pool 2, stageA/tr 2.
"""

import numpy as np
import ml_dtypes

import concourse.bacc as bacc
import concourse.bass as bass
import concourse.mybir as mybir
from concourse import tile
from concourse.bass import broadcast_tensor_aps
from concourse.bass_utils import run_bass_kernel_spmd

BF16 = ml_dtypes.bfloat16

B, T, C, H, W = 2, 4, 256, 128, 128
HEADS, DH = 8, 32
SCALE = DH ** (-0.5)
F = 3 * C  # 768
WBLK = 16
NBLK = W // WBLK  # 8
RBLK = H * WBLK  # 2048 block columns, (w, h) ordered
DT_B = mybir.dt.bfloat16
DT_F = mybir.dt.float32
AF = mybir.ActivationFunctionType
MUL = mybir.AluOpType.mult
DIV = mybir.AluOpType.divide
ADD = mybir.AluOpType.add

# stage-A copy split: this hq chunk goes to Act, rest to DVE (balance)
STAGEA_ACT = (1,)


def build_program():
    nc = bacc.Bacc("TRN2")

    # x_wt: host-transposed [C, W, H]
    x_wt = nc.dram_tensor("x_wt", [C, W, H], DT_B, kind="ExternalInput")
    # packed constants: [w1 768 | w2 768 | wo1 256 | wo2 256 | expbt 1024 | id 128]
    cb16 = nc.dram_tensor("cb16", [128, 3200], DT_B, kind="ExternalInput")
    # packed f32 biases: [bqk 4 | bout2 2]
    cf32 = nc.dram_tensor("cf32", [128, 6], DT_F, kind="ExternalInput")
    out_bt = nc.dram_tensor("out_bt", [C, H, W], DT_F, kind="ExternalOutput")

    with tile.TileContext(nc) as tc:
        with (
            tc.tile_pool(name="const", bufs=1) as cpool,
            tc.tile_pool(name="xt", bufs=1) as xt_pool,
            tc.tile_pool(name="qk", bufs=8) as qk_pool,
            tc.tile_pool(name="vp", bufs=1) as v_pool,
            tc.tile_pool(name="ex", bufs=4) as ex_pool,
            tc.tile_pool(name="at", bufs=4) as at_pool,
            tc.tile_pool(name="yn", bufs=6) as yn_pool,
            tc.tile_pool(name="yt", bufs=2) as yt_pool,
            tc.tile_pool(name="outp", bufs=2) as out_pool,
            tc.tile_pool(name="pssc", bufs=2, space="PSUM") as ps_sc,
            tc.tile_pool(name="psvy", bufs=2, space="PSUM") as ps_vy,
            tc.tile_pool(name="psms", bufs=2, space="PSUM") as ps_ms,
        ):
            # ---- constants (2 packed DMAs) ----
            cb = cpool.tile([128, 3200], DT_B, tag="cb16")
            nc.sync.dma_start(out=cb[:], in_=cb16[:])
            cf = cpool.tile([128, 6], DT_F, tag="cf32")
            nc.sync.dma_start(out=cf[:], in_=cf32[:])
            w1 = cb[:, 0:768]
            w2 = cb[:, 768:1536]
            wo1 = cb[:, 1536:1792]
            wo2 = cb[:, 1792:2048]
            eb_sb = cb[:, 2048:3072]
            id_sb = cb[:, 3072:3200]
            bqk_sb = cf[:, 0:4]
            bout_sb = cf[:, 4:6]

            wq = (w1, w2)

            # fence the tiny const loads off the downstream sync graph
            tc.strict_bb_all_engine_barrier()

            # ---- resident X^T [c, (w, h)]: 8 w-sixteenth chunks; block wb
            # depends only on chunk wb ----
            xt_all = xt_pool.tile([128, 2 * H * W], DT_B, tag="xt")
            xt_v = xt_all[:].rearrange("p (a f) -> p a f", a=2)
            for ck in range(8):
                eng = nc.sync if ck % 2 == 0 else nc.scalar
                eng.dma_start(
                    out=xt_v[:, :, ck * 2048 : (ck + 1) * 2048],
                    in_=x_wt[:, ck * 16 : (ck + 1) * 16, :].rearrange(
                        "(a c) w h -> c a (w h)", a=2
                    ),
                )
            # persistent per-line V tiles; ones columns written once
            v_tiles = []
            for j in range(WBLK):
                vt = v_pool.tile([128, HEADS * 33], DT_B, tag=f"vt{j}")
                nc.vector.memset(
                    vt[:].rearrange("p (a b) -> p a b", a=HEADS, b=33)[:, :, 32],
                    1.0,
                )
                v_tiles.append(vt)

            # (w, h)-major views [c, w, h] of the two c-halves
            xv0 = xt_all[:, : H * W].rearrange("p (w h) -> p w h", w=W)
            xv1 = xt_all[:, H * W :].rearrange("p (w h) -> p w h", w=W)
            xvs = (xv0, xv1)

            # ================= building blocks =================
            def sa_alloc():
                tiles = []
                for _ in range(4):
                    qt = qk_pool.tile([128, RBLK], DT_B, tag="qkt")
                    tiles.append(qt)
                return tiles

            def sa_chunk(wb, tiles, ft, hq):
                w0 = wb * WBLK
                qtv = tiles[ft][:].rearrange("p (w h) -> p w h", w=WBLK)
                ps = ps_ms.tile([128, 512], DT_F, tag="ms")
                for ct in range(2):
                    nc.tensor.matmul(
                        ps[:],
                        lhsT=wq[ct][:, ft * 128 : (ft + 1) * 128],
                        rhs=xvs[ct][:, w0 : w0 + WBLK, hq * 32 : (hq + 1) * 32],
                        start=(ct == 0),
                        stop=(ct == 1),
                    )
                dst = qtv[:, :, hq * 32 : (hq + 1) * 32]
                src = ps[:].rearrange("p (w h) -> p w h", w=WBLK)
                if hq in STAGEA_ACT:
                    nc.scalar.activation(
                        dst, src, AF.Identity, bias=bqk_sb[:, ft : ft + 1]
                    )
                else:
                    nc.vector.tensor_scalar(
                        out=dst, in0=src,
                        scalar1=bqk_sb[:, ft : ft + 1],
                        scalar2=None, op0=ADD,
                    )

            def v_line(wb, j):
                w = wb * WBLK + j
                ps = ps_vy.tile([128, 256], DT_F, tag="vy")
                for ct in range(2):
                    nc.tensor.matmul(
                        ps[:],
                        lhsT=xvs[ct][:, w, :],
                        rhs=wq[ct][:, 512:768],
                        start=(ct == 0),
                        stop=(ct == 1),
                    )
                vt3 = v_tiles[j][:].rearrange("p (a b) -> p a b", a=HEADS, b=33)
                nc.vector.tensor_copy(
                    vt3[:, :, 0:32],
                    ps[:].rearrange("p (a b) -> p a b", a=HEADS, b=32),
                )

            def scores_exp_eb(qk_tiles, p):
                """scores -> exp -> attnw for lines (2p, 2p+1).

                psum tile tt: bank0 = heads (2tt, 2tt+4) both lines (one PE
                row band), bank1 = heads (2tt+1, 2tt+5).
                """
                ats = []
                for tt in range(2):
                    ps = ps_sc.tile([128, 1024], DT_F, tag="sc")
                    for g in range(2):
                        hb = tt * 2 + g
                        r0 = hb * 32
                        for jj in range(2):
                            jc = slice((2 * p + jj) * 128, (2 * p + jj + 1) * 128)
                            for hh in range(2):  # head = hb + 4*hh
                                nc.tensor.matmul(
                                    ps[:, g * 512 + jj * 256 + hh * 128 :
                                       g * 512 + jj * 256 + (hh + 1) * 128],
                                    lhsT=qk_tiles[2 + hh][r0 : r0 + 32, jc],
                                    rhs=qk_tiles[hh][r0 : r0 + 32, jc],
                                    start=True,
                                    stop=True,
                                    tile_position=(r0, 0),
                                )
                    ex = ex_pool.tile([128, 1024], DT_B, tag="ex")
                    nc.scalar.activation(ex[:], ps[:], AF.Exp, scale=SCALE)
                    at = at_pool.tile([128, 1024], DT_B, tag="at")
                    a3 = at[:].rearrange("p (g j c) -> p g j c", g=2, j=2)
                    e3 = ex[:].rearrange("p (g j c) -> p g j c", g=2, j=2)
                    b3 = eb_sb[:, tt * 512 : (tt + 1) * 512].rearrange(
                        "p (g c) -> p g c", g=2
                    ).rearrange("p g (o c) -> p g o c", o=1)
                    i0, i1 = broadcast_tensor_aps(e3, b3)
                    nc.gpsimd.tensor_tensor(a3, i0, i1, MUL)
                    ats.append(at)
                return ats

            def at_off(h, jj):
                g = (h % 4) % 2
                return g * 512 + jj * 256 + (h // 4) * 128

            def av_yn(ats, p):
                """AV matmuls + normalize (divide) for lines (2p, 2p+1).
                Returns the two normalized-yn SBUF tiles."""
                yns = []
                for jj in range(2):
                    j = (2 * p + jj) % WBLK
                    y_ps = ps_vy.tile([128, HEADS * 33], DT_F, tag="vy")
                    for h in range(HEADS):
                        half = (h % 4) // 2
                        nc.tensor.matmul(
                            y_ps[:, h * 33 : h * 33 + 33],
                            lhsT=ats[half][:, at_off(h, jj) : at_off(h, jj) + 128],
                            rhs=v_tiles[j][:, h * 33 : h * 33 + 33],
                            start=True,
                            stop=True,
                        )
                    y3 = y_ps[:].rearrange("p (a b) -> p a b", a=HEADS, b=33)
                    yn = yn_pool.tile([128, C], DT_B, tag="yn")
                    i0, i1 = broadcast_tensor_aps(y3[:, :, 0:32], y3[:, :, 32:33])
                    nc.vector.tensor_tensor(
                        yn[:].rearrange("p (a b) -> p a b", a=HEADS, b=32),
                        i0, i1, DIV,
                    )
                    yns.append(yn)
                return yns

            def tail_tr(yns, p, yt):
                """transpose + store Y^T into the block buffer."""
                for jj in range(2):
                    j = (2 * p + jj) % WBLK
                    jc = slice(j * 128, (j + 1) * 128)
                    yn = yns[jj]
                    tr = ps_ms.tile([128, 256], DT_B, tag="ms")
                    nc.tensor.transpose(tr[:, 0:128], yn[:, 0:128], id_sb[:])
                    nc.tensor.transpose(tr[:, 128:256], yn[:, 128:256], id_sb[:])
                    nc.vector.tensor_copy(
                        yt[:].rearrange("p (a l) -> p a l", a=2)[:, :, jc],
                        tr[:].rearrange("p (a l) -> p a l", a=2),
                    )

            def proj_tile(wb, yt, i, ots):
                """one out-projection psum tile (i in 0..8) + biased copy;
                fires the half-output DMA after tiles 3 and 7."""
                ct, ch = i // 4, i % 4
                if ch == 0:
                    ot = out_pool.tile([128, RBLK], DT_F, tag="ot")
                    ots[ct] = ot
                ot = ots[ct]
                otv = ot[:].rearrange("p (h w) -> p h w", w=WBLK)
                ps = ps_sc.tile([128, 512], DT_F, tag="sc")
                nc.tensor.matmul(
                    ps[:],
                    lhsT=wo1[:, ct * 128 : (ct + 1) * 128],
                    rhs=yt[:, ch * 512 : (ch + 1) * 512],
                    start=True, stop=False,
                )
                nc.tensor.matmul(
                    ps[:],
                    lhsT=wo2[:, ct * 128 : (ct + 1) * 128],
                    rhs=yt[:, RBLK + ch * 512 : RBLK + (ch + 1) * 512],
                    start=False, stop=True,
                )
                # psum cols are (w 4, h 128); write reordered to (h, w)
                nc.scalar.activation(
                    otv[:, :, ch * 4 : (ch + 1) * 4],
                    ps[:].rearrange("p (w h) -> p h w", w=4),
                    AF.Identity,
                    bias=bout_sb[:, ct : ct + 1],
                )
                if ch == 3:
                    w0 = wb * WBLK
                    nc.sync.dma_start(
                        out=out_bt[ct * 128 : (ct + 1) * 128, :, w0 : w0 + WBLK],
                        in_=ot[:],
                    )

            # ================= flat slot pipeline =================
            NPAIR = NBLK * NBLK  # 64
            qk_by = {}
            yts = {}
            at_state = {}
            yn_state = {}
            ots_by = {}

            # prologue: block 0 stage-A (x-chunk arrival order) + V lines
            qk_by[0] = sa_alloc()
            for hq in range(4):
                for ft in range(4):
                    sa_chunk(0, qk_by[0], ft, hq)
            for j in range(WBLK):
                v_line(0, j)
            yts[0] = yt_pool.tile([128, 2 * RBLK], DT_B, tag="yt", name="yt0")

            for s in range(NPAIR + 2):
                if s < NPAIR:
                    wb, p = divmod(s, NBLK)
                    if p == 0 and wb > 0:
                        yts[wb] = yt_pool.tile([128, 2 * RBLK], DT_B, tag="yt", name=f"yt{wb}")
                    at_state[s] = scores_exp_eb(qk_by[wb], p)
                if s >= 1 and s - 1 < NPAIR:
                    wb1, p1 = divmod(s - 1, NBLK)
                    yn_state[s - 1] = av_yn(at_state.pop(s - 1), p1)
                if s >= 2 and s - 2 < NPAIR:
                    wb2, p2 = divmod(s - 2, NBLK)
                    tail_tr(yn_state.pop(s - 2), p2, yts[wb2])
                if s >= NPAIR:
                    continue
                # ---- phase-scheduled fillers ----
                # prev block's projection at p in 2..5 (2 tiles each)
                if wb >= 1 and 2 <= p <= 5:
                    if p == 2:
                        ots_by[wb - 1] = {}
                    for k in range(2):
                        proj_tile(wb - 1, yts[wb - 1], (p - 2) * 2 + k,
                                  ots_by[wb - 1])
                # next block's stage A at p in 4..7 (4 chunks each)
                if wb < NBLK - 1 and 4 <= p <= 7:
                    if p == 4:
                        qk_by[wb + 1] = sa_alloc()
                    for k in range(4):
                        ft, hq = divmod((p - 4) * 4 + k, 4)
                        sa_chunk(wb + 1, qk_by[wb + 1], ft, hq)
                # next block's V lines: p in 5..7 -> lines 0..11, and the
                # last 4 lines early in the next block (after their WAR
                # pairs drain)
                if wb < NBLK - 1 and 5 <= p <= 7:
                    for j in range(4 * (p - 5), 4 * (p - 5) + 4):
                        v_line(wb + 1, j)
                if wb >= 1 and p == 0:
                    for j in range(12, 16):
                        v_line(wb, j)

            # epilogue: last block's projection
            ots_by[NBLK - 1] = {}
            for i in range(8):
                proj_tile(NBLK - 1, yts[NBLK - 1], i, ots_by[NBLK - 1])

    nc.compile()
    return nc


_NC = None


def _get_nc():
    global _NC
    if _NC is None:
        _NC = build_program()
    return _NC


def _prep_small(rel_bias, Wqkv, bqkv, Wout, bout):
    # bf16 blob: [w1 768 | w2 768 | wo1 256 | wo2 256 | expbt 1024 | id 128]
    w12 = Wqkv.reshape(2, 128, F)
    wo12 = Wout.reshape(2, 128, C)
    expbt_a = np.exp(rel_bias.transpose(0, 2, 1))  # [hd, m, l]
    # head order (0,4),(1,5),(2,6),(3,7): pair (hd, hd+4) shares a PE row
    # band, so the pair's scores can share one PSUM bank safely
    expbt_a = expbt_a[[0, 4, 1, 5, 2, 6, 3, 7]]
    eb = expbt_a.transpose(1, 0, 2).reshape(128, HEADS * 128)  # [m, (hd, l)]
    cb16 = np.concatenate(
        [w12[0], w12[1], wo12[0], wo12[1], eb, np.eye(128, dtype=np.float32)],
        axis=1,
    ).astype(BF16)
    # raw biases (softmax scale folded into the exp activation's scale)
    bqk_a = np.stack(
        [bqkv[0:128], bqkv[128:256], bqkv[256:384], bqkv[384:512]],
        axis=1,
    )
    bout2_a = (bout + bqkv[512:] @ Wout).reshape(2, 128).T
    cf32 = np.concatenate([bqk_a, bout2_a], axis=1).astype(np.float32)
    return {"cb16": np.ascontiguousarray(cb16), "cf32": np.ascontiguousarray(cf32)}


def _run(x, rel_bias, Wqkv, bqkv, Wout, bout, **spmd_kwargs):
    x = np.asarray(x, dtype=np.float32)
    small = _prep_small(
        np.asarray(rel_bias, np.float32),
        np.asarray(Wqkv, np.float32),
        np.asarray(bqkv, np.float32),
        np.asarray(Wout, np.float32),
        np.asarray(bout, np.float32),
    )
    nc = _get_nc()
    core_ids = list(range(8))
    in_maps = []
    for i in core_ids:
        b, t = divmod(i, T)
        m = dict(small)
        # host transpose to [C, W, H] (w-major chunks)
        m["x_wt"] = np.ascontiguousarray(
            x[b, t].transpose(0, 2, 1)
        ).astype(BF16)
        in_maps.append(m)
    res = run_bass_kernel_spmd(nc, in_maps, core_ids, **spmd_kwargs)
    out = np.empty((B, T, C, H, W), np.float32)
    for i in core_ids:
        b, t = divmod(i, T)
        out[b, t] = res.results[i]["out_bt"]
    return out, res


def kernel(x, rel_bias, Wqkv, bqkv, Wout, bout):
    out, _ = _run(x, rel_bias, Wqkv, bqkv, Wout, bout)
    return out


# revision 15
# speedup vs baseline: 1.3134x; 1.0665x over previous
"""AxialAttention (axis=height) Trainium2 Bass kernel, v4.

Problem: x [B=2,T=4,C=256,H=128,W=128] f32. Lines run along H; N = B*T*W
independent sequences of length L=H=128 with C=256 channels, 8 heads x 32.
Sharding: one (b,t) pair per core (8 cores == B*T).

Engine-balance + pipeline design (baseline was Activation-bound at 93%):
  - x is host-transposed to [C, W, H] so each w-block's stage-A/V work
    depends on exactly one of the 8 streamed x DMA chunks (fast start).
  - scores PSUM tiles are [128, 1024] (2 banks) per (2 lines, 2 head-pair
    groups): bank0 holds heads (g, g+4) of both lines -> one PE row band
    per bank (concurrent-matmul-safe), and ONE exp activation covers 1024
    cols, amortizing the Act engine's access-init (~185ns/op).
  - rel_bias applied multiplicatively post-exp on the otherwise-idle
    GPSIMD/Pool engine (SBUF-only engine; it cannot touch PSUM).
  - softmax normalization is a single TensorTensor DIVIDE by the
    denominator column (ones-column trick in the AV matmul).
  - qkv biases ride the PSUM->SBUF copies (Act activation bias / DVE
    tensor_scalar add); softmax scale is folded into exp's scale; bv is
    folded into bout on the host.
  - flat 64-slot software pipeline (8 blocks x 8 line-pairs): slot s does
    scores/exp/ebmul(s), AV+normalize(s-1), transpose+store(s-2), plus
    phase-scheduled filler work (prev block's out-projection, next block's
    stage-A/V) to keep all queues deep without PSUM over-subscription.
PSUM (8 banks): scores+proj pool 2x[128,1024]=4, V/Y pool 2, stageA/tr 2.
"""

import numpy as np
import ml_dtypes

import concourse.bacc as bacc
import concourse.bass as bass
import concourse.mybir as mybir
from concourse import tile
from concourse.bass import broadcast_tensor_aps
from concourse.bass_utils import run_bass_kernel_spmd

BF16 = ml_dtypes.bfloat16

B, T, C, H, W = 2, 4, 256, 128, 128
HEADS, DH = 8, 32
SCALE = DH ** (-0.5)
F = 3 * C  # 768
WBLK = 16
NBLK = W // WBLK  # 8
RBLK = H * WBLK  # 2048 block columns, (w, h) ordered
DT_B = mybir.dt.bfloat16
DT_F = mybir.dt.float32
AF = mybir.ActivationFunctionType
MUL = mybir.AluOpType.mult
DIV = mybir.AluOpType.divide
ADD = mybir.AluOpType.add

# stage-A copy split: this hq chunk goes to Act, rest to DVE (balance)
STAGEA_ACT = (1,)


def build_program():
    nc = bacc.Bacc("TRN2")

    # x_wt: host-transposed [C, W, H]
    x_wt = nc.dram_tensor("x_wt", [C, W, H], DT_B, kind="ExternalInput")
    # packed constants: [w1 768 | w2 768 | wo1 256 | wo2 256 | expbt 1024 | id 128]
    cb16 = nc.dram_tensor("cb16", [128, 3200], DT_B, kind="ExternalInput")
    # packed f32 biases: [bqk 4 | bout2 2]
    cf32 = nc.dram_tensor("cf32", [128, 6], DT_F, kind="ExternalInput")
    out_bt = nc.dram_tensor("out_bt", [C, H, W], DT_F, kind="ExternalOutput")

    with tile.TileContext(nc) as tc:
        with (
            tc.tile_pool(name="const", bufs=1) as cpool,
            tc.tile_pool(name="xt", bufs=1) as xt_pool,
            tc.tile_pool(name="qk", bufs=8) as qk_pool,
            tc.tile_pool(name="vp", bufs=1) as v_pool,
            tc.tile_pool(name="ex", bufs=4) as ex_pool,
            tc.tile_pool(name="at", bufs=4) as at_pool,
            tc.tile_pool(name="yn", bufs=6) as yn_pool,
            tc.tile_pool(name="yt", bufs=2) as yt_pool,
            tc.tile_pool(name="outp", bufs=3) as out_pool,
            tc.tile_pool(name="pssc", bufs=2, space="PSUM") as ps_sc,
            tc.tile_pool(name="psvy", bufs=2, space="PSUM") as ps_vy,
            tc.tile_pool(name="psms", bufs=2, space="PSUM") as ps_ms,
        ):
            # ---- constants; wq first (gates the first stage-A matmuls) ----
            cb = cpool.tile([128, 3200], DT_B, tag="cb16")
            nc.scalar.dma_start(out=cb[:, 0:1536], in_=cb16[:, 0:1536])
            cf = cpool.tile([128, 6], DT_F, tag="cf32")
            nc.sync.dma_start(out=cf[:], in_=cf32[:])
            nc.sync.dma_start(out=cb[:, 1536:3200], in_=cb16[:, 1536:3200])
            w1 = cb[:, 0:768]
            w2 = cb[:, 768:1536]
            wo1 = cb[:, 1536:1792]
            wo2 = cb[:, 1792:2048]
            eb_sb = cb[:, 2048:3072]
            id_sb = cb[:, 3072:3200]
            bqk_sb = cf[:, 0:4]
            bout_sb = cf[:, 4:6]

            wq = (w1, w2)

            # fence the tiny const loads off the downstream sync graph
            tc.strict_bb_all_engine_barrier()

            # ---- resident X^T [c, (w, h)]: 8 w-sixteenth chunks; block wb
            # depends only on chunk wb ----
            xt_all = xt_pool.tile([128, 2 * H * W], DT_B, tag="xt")
            xt_v = xt_all[:].rearrange("p (a f) -> p a f", a=2)
            for ck in range(8):
                eng = nc.sync if ck % 2 == 0 else nc.scalar
                eng.dma_start(
                    out=xt_v[:, :, ck * 2048 : (ck + 1) * 2048],
                    in_=x_wt[:, ck * 16 : (ck + 1) * 16, :].rearrange(
                        "(a c) w h -> c a (w h)", a=2
                    ),
                )
            # persistent per-line V tiles; ones columns written once
            v_tiles = []
            for j in range(WBLK):
                vt = v_pool.tile([128, HEADS * 33], DT_B, tag=f"vt{j}")
                nc.vector.memset(
                    vt[:].rearrange("p (a b) -> p a b", a=HEADS, b=33)[:, :, 32],
                    1.0,
                )
                v_tiles.append(vt)

            # (w, h)-major views [c, w, h] of the two c-halves
            xv0 = xt_all[:, : H * W].rearrange("p (w h) -> p w h", w=W)
            xv1 = xt_all[:, H * W :].rearrange("p (w h) -> p w h", w=W)
            xvs = (xv0, xv1)

            # ================= building blocks =================
            def sa_alloc():
                tiles = []
                for _ in range(4):
                    qt = qk_pool.tile([128, RBLK], DT_B, tag="qkt")
                    tiles.append(qt)
                return tiles

            def sa_chunk(wb, tiles, ft, hq):
                w0 = wb * WBLK
                qtv = tiles[ft][:].rearrange("p (w h) -> p w h", w=WBLK)
                ps = ps_ms.tile([128, 512], DT_F, tag="ms")
                for ct in range(2):
                    nc.tensor.matmul(
                        ps[:],
                        lhsT=wq[ct][:, ft * 128 : (ft + 1) * 128],
                        rhs=xvs[ct][:, w0 : w0 + WBLK, hq * 32 : (hq + 1) * 32],
                        start=(ct == 0),
                        stop=(ct == 1),
                    )
                dst = qtv[:, :, hq * 32 : (hq + 1) * 32]
                src = ps[:].rearrange("p (w h) -> p w h", w=WBLK)
                if hq in STAGEA_ACT:
                    nc.scalar.activation(
                        dst, src, AF.Identity, bias=bqk_sb[:, ft : ft + 1]
                    )
                else:
                    nc.vector.tensor_scalar(
                        out=dst, in0=src,
                        scalar1=bqk_sb[:, ft : ft + 1],
                        scalar2=None, op0=ADD,
                    )

            def v_line(wb, j):
                w = wb * WBLK + j
                ps = ps_vy.tile([128, 256], DT_F, tag="vy")
                for ct in range(2):
                    nc.tensor.matmul(
                        ps[:],
                        lhsT=xvs[ct][:, w, :],
                        rhs=wq[ct][:, 512:768],
                        start=(ct == 0),
                        stop=(ct == 1),
                    )
                vt3 = v_tiles[j][:].rearrange("p (a b) -> p a b", a=HEADS, b=33)
                nc.vector.tensor_copy(
                    vt3[:, :, 0:32],
                    ps[:].rearrange("p (a b) -> p a b", a=HEADS, b=32),
                )

            def scores_exp_eb(qk_tiles, p):
                """scores -> exp -> attnw for lines (2p, 2p+1).

                psum tile tt: bank0 = heads (2tt, 2tt+4) both lines (one PE
                row band), bank1 = heads (2tt+1, 2tt+5).
                """
                ats = []
                for tt in range(2):
                    ps = ps_sc.tile([128, 1024], DT_F, tag="sc")
                    for g in range(2):
                        hb = tt * 2 + g
                        r0 = hb * 32
                        for jj in range(2):
                            jc = slice((2 * p + jj) * 128, (2 * p + jj + 1) * 128)
                            for hh in range(2):  # head = hb + 4*hh
                                nc.tensor.matmul(
                                    ps[:, g * 512 + jj * 256 + hh * 128 :
                                       g * 512 + jj * 256 + (hh + 1) * 128],
                                    lhsT=qk_tiles[2 + hh][r0 : r0 + 32, jc],
                                    rhs=qk_tiles[hh][r0 : r0 + 32, jc],
                                    start=True,
                                    stop=True,
                                    tile_position=(r0, 0),
                                )
                    ex = ex_pool.tile([128, 1024], DT_B, tag="ex")
                    nc.scalar.activation(ex[:], ps[:], AF.Exp, scale=SCALE)
                    at = at_pool.tile([128, 1024], DT_B, tag="at")
                    a3 = at[:].rearrange("p (g j c) -> p g j c", g=2, j=2)
                    e3 = ex[:].rearrange("p (g j c) -> p g j c", g=2, j=2)
                    b3 = eb_sb[:, tt * 512 : (tt + 1) * 512].rearrange(
                        "p (g c) -> p g c", g=2
                    ).rearrange("p g (o c) -> p g o c", o=1)
                    i0, i1 = broadcast_tensor_aps(e3, b3)
                    nc.gpsimd.tensor_tensor(a3, i0, i1, MUL)
                    ats.append(at)
                return ats

            def at_off(h, jj):
                g = (h % 4) % 2
                return g * 512 + jj * 256 + (h // 4) * 128

            def av_yn(ats, p):
                """AV matmuls + normalize (divide) for lines (2p, 2p+1).
                Returns the two normalized-yn SBUF tiles."""
                yns = []
                for jj in range(2):
                    j = (2 * p + jj) % WBLK
                    y_ps = ps_vy.tile([128, HEADS * 33], DT_F, tag="vy")
                    for h in range(HEADS):
                        half = (h % 4) // 2
                        nc.tensor.matmul(
                            y_ps[:, h * 33 : h * 33 + 33],
                            lhsT=ats[half][:, at_off(h, jj) : at_off(h, jj) + 128],
                            rhs=v_tiles[j][:, h * 33 : h * 33 + 33],
                            start=True,
                            stop=True,
                        )
                    y3 = y_ps[:].rearrange("p (a b) -> p a b", a=HEADS, b=33)
                    yn = yn_pool.tile([128, C], DT_B, tag="yn")
                    i0, i1 = broadcast_tensor_aps(y3[:, :, 0:32], y3[:, :, 32:33])
                    nc.vector.tensor_tensor(
                        yn[:].rearrange("p (a b) -> p a b", a=HEADS, b=32),
                        i0, i1, DIV,
                    )
                    yns.append(yn)
                return yns

            def tail_tr(yns, p, yt):
                """transpose + store Y^T into the block buffer."""
                for jj in range(2):
                    j = (2 * p + jj) % WBLK
                    jc = slice(j * 128, (j + 1) * 128)
                    yn = yns[jj]
                    tr = ps_ms.tile([128, 256], DT_B, tag="ms")
                    nc.tensor.transpose(tr[:, 0:128], yn[:, 0:128], id_sb[:])
                    nc.tensor.transpose(tr[:, 128:256], yn[:, 128:256], id_sb[:])
                    nc.vector.tensor_copy(
                        yt[:].rearrange("p (a l) -> p a l", a=2)[:, :, jc],
                        tr[:].rearrange("p (a l) -> p a l", a=2),
                    )

            def proj_tile(wb, yt, i, ots):
                """one out-projection psum tile (i in 0..8) + biased copy;
                fires the half-output DMA after tiles 3 and 7."""
                ct, ch = i // 4, i % 4
                if ch == 0:
                    ot = out_pool.tile([128, RBLK], DT_F, tag="ot")
                    ots[ct] = ot
                ot = ots[ct]
                otv = ot[:].rearrange("p (h w) -> p h w", w=WBLK)
                ps = ps_sc.tile([128, 512], DT_F, tag="sc")
                nc.tensor.matmul(
                    ps[:],
                    lhsT=wo1[:, ct * 128 : (ct + 1) * 128],
                    rhs=yt[:, ch * 512 : (ch + 1) * 512],
                    start=True, stop=False,
                )
                nc.tensor.matmul(
                    ps[:],
                    lhsT=wo2[:, ct * 128 : (ct + 1) * 128],
                    rhs=yt[:, RBLK + ch * 512 : RBLK + (ch + 1) * 512],
                    start=False, stop=True,
                )
                # psum cols are (w 4, h 128); write reordered to (h, w)
                nc.scalar.activation(
                    otv[:, :, ch * 4 : (ch + 1) * 4],
                    ps[:].rearrange("p (w h) -> p h w", w=4),
                    AF.Identity,
                    bias=bout_sb[:, ct : ct + 1],
                )
                if ch == 3:
                    w0 = wb * WBLK
                    nc.sync.dma_start(
                        out=out_bt[ct * 128 : (ct + 1) * 128, :, w0 : w0 + WBLK],
                        in_=ot[:],
                    )

            # ================= flat slot pipeline =================
            NPAIR = NBLK * NBLK  # 64
            qk_by = {}
            yts = {}
            at_state = {}
            yn_state = {}
            ots_by = {}

            # prologue: block 0 stage-A (x-chunk arrival order) + V lines
            qk_by[0] = sa_alloc()
            for hq in range(4):
                for ft in range(4):
                    sa_chunk(0, qk_by[0], ft, hq)
            for j in range(WBLK):
                v_line(0, j)
            yts[0] = yt_pool.tile([128, 2 * RBLK], DT_B, tag="yt", name="yt0")

            for s in range(NPAIR + 2):
                if s < NPAIR:
                    wb, p = divmod(s, NBLK)
                    if p == 0 and wb > 0:
                        yts[wb] = yt_pool.tile([128, 2 * RBLK], DT_B, tag="yt", name=f"yt{wb}")
                    at_state[s] = scores_exp_eb(qk_by[wb], p)
                if s >= 1 and s - 1 < NPAIR:
                    wb1, p1 = divmod(s - 1, NBLK)
                    yn_state[s - 1] = av_yn(at_state.pop(s - 1), p1)
                if s >= 2 and s - 2 < NPAIR:
                    wb2, p2 = divmod(s - 2, NBLK)
                    tail_tr(yn_state.pop(s - 2), p2, yts[wb2])
                if s >= NPAIR:
                    continue
                # ---- phase-scheduled fillers (thin bursts) ----
                # prev block's projection: ch = p at slots 0..3 (2 tiles/slot)
                if wb >= 1 and p <= 3:
                    if p == 0:
                        ots_by[wb - 1] = {}
                    for ct in range(2):
                        proj_tile(wb - 1, yts[wb - 1], ct * 4 + p,
                                  ots_by[wb - 1])
                # last block's projection pulled into its own tail slots
                if wb == NBLK - 1:
                    if p == 4:
                        ots_by[wb] = {}
                        proj_tile(wb, yts[wb], 0, ots_by[wb])
                        proj_tile(wb, yts[wb], 4, ots_by[wb])
                    elif p == 6:
                        proj_tile(wb, yts[wb], 1, ots_by[wb])
                        proj_tile(wb, yts[wb], 5, ots_by[wb])
                # next block's stage A: slots 1..6, counts 3,3,3,3,2,2
                if wb < NBLK - 1 and 1 <= p <= 6:
                    if p == 1:
                        qk_by[wb + 1] = sa_alloc()
                    base = [0, 3, 6, 9, 12, 14][p - 1]
                    cnt = [3, 3, 3, 3, 2, 2][p - 1]
                    for k in range(base, base + cnt):
                        ft, hq = divmod(k, 4)
                        sa_chunk(wb + 1, qk_by[wb + 1], ft, hq)
                # next block's V lines: 2 per slot at p=1..7, last 2 lines
                # early in the next block (after their WAR pairs drain)
                if wb < NBLK - 1 and 1 <= p <= 7:
                    for j in (2 * (p - 1), 2 * (p - 1) + 1):
                        v_line(wb + 1, j)
                if wb >= 1 and p == 0:
                    for j in (14, 15):
                        v_line(wb, j)

            # epilogue: last block's remaining projection (ct-major so the
            # first half-output DMA fires as early as possible)
            for i in (2, 3, 6, 7):
                proj_tile(NBLK - 1, yts[NBLK - 1], i, ots_by[NBLK - 1])

    nc.compile()
    return nc


_NC = None


def _get_nc():
    global _NC
    if _NC is None:
        _NC = build_program()
    return _NC


def _prep_small(rel_bias, Wqkv, bqkv, Wout, bout):
    # bf16 blob: [w1 768 | w2 768 | wo1 256 | wo2 256 | expbt 1024 | id 128]
    w12 = Wqkv.reshape(2, 128, F)
    wo12 = Wout.reshape(2, 128, C)
    expbt_a = np.exp(rel_bias.transpose(0, 2, 1))  # [hd, m, l]
    # head order (0,4),(1,5),(2,6),(3,7): pair (hd, hd+4) shares a PE row
    # band, so the pair's scores can share one PSUM bank safely
    expbt_a = expbt_a[[0, 4, 1, 5, 2, 6, 3, 7]]
    eb = expbt_a.transpose(1, 0, 2).reshape(128, HEADS * 128)  # [m, (hd, l)]
    cb16 = np.concatenate(
        [w12[0], w12[1], wo12[0], wo12[1], eb, np.eye(128, dtype=np.float32)],
        axis=1,
    ).astype(BF16)
    # raw biases (softmax scale folded into the exp activation's scale)
    bqk_a = np.stack(
        [bqkv[0:128], bqkv[128:256], bqkv[256:384], bqkv[384:512]],
        axis=1,
    )
    bout2_a = (bout + bqkv[512:] @ Wout).reshape(2, 128).T
    cf32 = np.concatenate([bqk_a, bout2_a], axis=1).astype(np.float32)
    return {"cb16": np.ascontiguousarray(cb16), "cf32": np.ascontiguousarray(cf32)}


def _run(x, rel_bias, Wqkv, bqkv, Wout, bout, **spmd_kwargs):
    x = np.asarray(x, dtype=np.float32)
    small = _prep_small(
        np.asarray(rel_bias, np.float32),
        np.asarray(Wqkv, np.float32),
        np.asarray(bqkv, np.float32),
        np.asarray(Wout, np.float32),
        np.asarray(bout, np.float32),
    )
    nc = _get_nc()
    core_ids = list(range(8))
    in_maps = []
    for i in core_ids:
        b, t = divmod(i, T)
        m = dict(small)
        # host transpose to [C, W, H] (w-major chunks)
        m["x_wt"] = np.ascontiguousarray(
            x[b, t].transpose(0, 2, 1)
        ).astype(BF16)
        in_maps.append(m)
    res = run_bass_kernel_spmd(nc, in_maps, core_ids, **spmd_kwargs)
    out = np.empty((B, T, C, H, W), np.float32)
    for i in core_ids:
        b, t = divmod(i, T)
        out[b, t] = res.results[i]["out_bt"]
    return out, res


def kernel(x, rel_bias, Wqkv, bqkv, Wout, bout):
    out, _ = _run(x, rel_bias, Wqkv, bqkv, Wout, bout)
    return out


# revision 16
# speedup vs baseline: 1.3143x; 1.0007x over previous
"""AxialAttention (axis=height) Trainium2 Bass kernel, v4.

Problem: x [B=2,T=4,C=256,H=128,W=128] f32. Lines run along H; N = B*T*W
independent sequences of length L=H=128 with C=256 channels, 8 heads x 32.
Sharding: one (b,t) pair per core (8 cores == B*T).

Engine-balance + pipeline design (baseline was Activation-bound at 93%):
  - x is host-transposed to [C, W, H] so each w-block's stage-A/V work
    depends on exactly one of the 8 streamed x DMA chunks (fast start).
  - scores PSUM tiles are [128, 1024] (2 banks) per (2 lines, 2 head-pair
    groups): bank0 holds heads (g, g+4) of both lines -> one PE row band
    per bank (concurrent-matmul-safe), and ONE exp activation covers 1024
    cols, amortizing the Act engine's access-init (~185ns/op).
  - rel_bias applied multiplicatively post-exp on the otherwise-idle
    GPSIMD/Pool engine (SBUF-only engine; it cannot touch PSUM).
  - softmax normalization is a single TensorTensor DIVIDE by the
    denominator column (ones-column trick in the AV matmul).
  - qkv biases ride the PSUM->SBUF copies (Act activation bias / DVE
    tensor_scalar add); softmax scale is folded into exp's scale; bv is
    folded into bout on the host.
  - flat 64-slot software pipeline (8 blocks x 8 line-pairs): slot s does
    scores/exp/ebmul(s), AV+normalize(s-1), transpose+store(s-2), plus
    phase-scheduled filler work (prev block's out-projection, next block's
    stage-A/V) to keep all queues deep without PSUM over-subscription.
PSUM (8 banks): scores+proj pool 2x[128,1024]=4, V/Y pool 2, stageA/tr 2.
"""

import numpy as np
import ml_dtypes

import concourse.bacc as bacc
import concourse.bass as bass
import concourse.mybir as mybir
from concourse import tile
from concourse.bass import broadcast_tensor_aps
from concourse.bass_utils import run_bass_kernel_spmd

BF16 = ml_dtypes.bfloat16

B, T, C, H, W = 2, 4, 256, 128, 128
HEADS, DH = 8, 32
SCALE = DH ** (-0.5)
F = 3 * C  # 768
WBLK = 16
NBLK = W // WBLK  # 8
RBLK = H * WBLK  # 2048 block columns, (w, h) ordered
DT_B = mybir.dt.bfloat16
DT_F = mybir.dt.float32
AF = mybir.ActivationFunctionType
MUL = mybir.AluOpType.mult
DIV = mybir.AluOpType.divide
ADD = mybir.AluOpType.add

# stage-A copy split: this hq chunk goes to Act, rest to DVE (balance)
STAGEA_ACT = (1,)


def build_program():
    nc = bacc.Bacc("TRN2")

    # x_wt: host-transposed [C, W, H]
    x_wt = nc.dram_tensor("x_wt", [C, W, H], DT_B, kind="ExternalInput")
    # packed constants: [w1 768 | w2 768 | wo1 256 | wo2 256 | expbt 1024 | id 128]
    cb16 = nc.dram_tensor("cb16", [128, 3200], DT_B, kind="ExternalInput")
    # packed f32 biases: [bqk 4 | bout2 2]
    cf32 = nc.dram_tensor("cf32", [128, 6], DT_F, kind="ExternalInput")
    # out is stored w-major [C, W, H]; the host transposes back to [C, H, W]
    out_wt = nc.dram_tensor("out_wt", [C, W, H], DT_F, kind="ExternalOutput")

    with tile.TileContext(nc) as tc:
        with (
            tc.tile_pool(name="const", bufs=1) as cpool,
            tc.tile_pool(name="xt", bufs=1) as xt_pool,
            tc.tile_pool(name="qk", bufs=8) as qk_pool,
            tc.tile_pool(name="vp", bufs=1) as v_pool,
            tc.tile_pool(name="ex", bufs=4) as ex_pool,
            tc.tile_pool(name="at", bufs=4) as at_pool,
            tc.tile_pool(name="yn", bufs=6) as yn_pool,
            tc.tile_pool(name="yt", bufs=2) as yt_pool,
            tc.tile_pool(name="outp", bufs=3) as out_pool,
            tc.tile_pool(name="pssc", bufs=2, space="PSUM") as ps_sc,
            tc.tile_pool(name="psvy", bufs=2, space="PSUM") as ps_vy,
            tc.tile_pool(name="psms", bufs=2, space="PSUM") as ps_ms,
        ):
            # ---- constants; wq first (gates the first stage-A matmuls) ----
            cb = cpool.tile([128, 3200], DT_B, tag="cb16")
            nc.sync.dma_start(out=cb[:, 0:1536], in_=cb16[:, 0:1536])
            cf = cpool.tile([128, 6], DT_F, tag="cf32")
            nc.sync.dma_start(out=cf[:], in_=cf32[:])
            nc.sync.dma_start(out=cb[:, 1536:3200], in_=cb16[:, 1536:3200])
            w1 = cb[:, 0:768]
            w2 = cb[:, 768:1536]
            wo1 = cb[:, 1536:1792]
            wo2 = cb[:, 1792:2048]
            eb_sb = cb[:, 2048:3072]
            id_sb = cb[:, 3072:3200]
            bqk_sb = cf[:, 0:4]
            bout_sb = cf[:, 4:6]

            wq = (w1, w2)

            # fence the tiny const loads off the downstream sync graph
            tc.strict_bb_all_engine_barrier()

            # ---- resident X^T [c, (w, h)]: 8 w-sixteenth chunks; block wb
            # depends only on chunk wb ----
            xt_all = xt_pool.tile([128, 2 * H * W], DT_B, tag="xt")
            xt_v = xt_all[:].rearrange("p (a f) -> p a f", a=2)
            for ck in range(8):
                nc.sync.dma_start(
                    out=xt_v[:, :, ck * 2048 : (ck + 1) * 2048],
                    in_=x_wt[:, ck * 16 : (ck + 1) * 16, :].rearrange(
                        "(a c) w h -> c a (w h)", a=2
                    ),
                )
            # persistent per-line V tiles; ones columns written once
            v_tiles = []
            for j in range(WBLK):
                vt = v_pool.tile([128, HEADS * 33], DT_B, tag=f"vt{j}")
                nc.vector.memset(
                    vt[:].rearrange("p (a b) -> p a b", a=HEADS, b=33)[:, :, 32],
                    1.0,
                )
                v_tiles.append(vt)

            # (w, h)-major views [c, w, h] of the two c-halves
            xv0 = xt_all[:, : H * W].rearrange("p (w h) -> p w h", w=W)
            xv1 = xt_all[:, H * W :].rearrange("p (w h) -> p w h", w=W)
            xvs = (xv0, xv1)

            # ================= building blocks =================
            def sa_alloc():
                tiles = []
                for _ in range(4):
                    qt = qk_pool.tile([128, RBLK], DT_B, tag="qkt")
                    tiles.append(qt)
                return tiles

            def sa_chunk(wb, tiles, ft, hq):
                w0 = wb * WBLK
                qtv = tiles[ft][:].rearrange("p (w h) -> p w h", w=WBLK)
                ps = ps_ms.tile([128, 512], DT_F, tag="ms")
                for ct in range(2):
                    nc.tensor.matmul(
                        ps[:],
                        lhsT=wq[ct][:, ft * 128 : (ft + 1) * 128],
                        rhs=xvs[ct][:, w0 : w0 + WBLK, hq * 32 : (hq + 1) * 32],
                        start=(ct == 0),
                        stop=(ct == 1),
                    )
                dst = qtv[:, :, hq * 32 : (hq + 1) * 32]
                src = ps[:].rearrange("p (w h) -> p w h", w=WBLK)
                if hq in STAGEA_ACT:
                    nc.scalar.activation(
                        dst, src, AF.Identity, bias=bqk_sb[:, ft : ft + 1]
                    )
                else:
                    nc.vector.tensor_scalar(
                        out=dst, in0=src,
                        scalar1=bqk_sb[:, ft : ft + 1],
                        scalar2=None, op0=ADD,
                    )

            def v_line(wb, j):
                w = wb * WBLK + j
                ps = ps_vy.tile([128, 256], DT_F, tag="vy")
                for ct in range(2):
                    nc.tensor.matmul(
                        ps[:],
                        lhsT=xvs[ct][:, w, :],
                        rhs=wq[ct][:, 512:768],
                        start=(ct == 0),
                        stop=(ct == 1),
                    )
                vt3 = v_tiles[j][:].rearrange("p (a b) -> p a b", a=HEADS, b=33)
                nc.vector.tensor_copy(
                    vt3[:, :, 0:32],
                    ps[:].rearrange("p (a b) -> p a b", a=HEADS, b=32),
                )

            def scores_exp_eb(qk_tiles, p):
                """scores -> exp -> attnw for lines (2p, 2p+1).

                psum tile tt: bank0 = heads (2tt, 2tt+4) both lines (one PE
                row band), bank1 = heads (2tt+1, 2tt+5).
                """
                ats = []
                for tt in range(2):
                    ps = ps_sc.tile([128, 1024], DT_F, tag="sc")
                    for g in range(2):
                        hb = tt * 2 + g
                        r0 = hb * 32
                        for jj in range(2):
                            jc = slice((2 * p + jj) * 128, (2 * p + jj + 1) * 128)
                            for hh in range(2):  # head = hb + 4*hh
                                nc.tensor.matmul(
                                    ps[:, g * 512 + jj * 256 + hh * 128 :
                                       g * 512 + jj * 256 + (hh + 1) * 128],
                                    lhsT=qk_tiles[2 + hh][r0 : r0 + 32, jc],
                                    rhs=qk_tiles[hh][r0 : r0 + 32, jc],
                                    start=True,
                                    stop=True,
                                    tile_position=(r0, 0),
                                )
                    ex = ex_pool.tile([128, 1024], DT_B, tag="ex")
                    nc.scalar.activation(ex[:], ps[:], AF.Exp, scale=SCALE)
                    at = at_pool.tile([128, 1024], DT_B, tag="at")
                    a3 = at[:].rearrange("p (g j c) -> p g j c", g=2, j=2)
                    e3 = ex[:].rearrange("p (g j c) -> p g j c", g=2, j=2)
                    b3 = eb_sb[:, tt * 512 : (tt + 1) * 512].rearrange(
                        "p (g c) -> p g c", g=2
                    ).rearrange("p g (o c) -> p g o c", o=1)
                    i0, i1 = broadcast_tensor_aps(e3, b3)
                    nc.gpsimd.tensor_tensor(a3, i0, i1, MUL)
                    ats.append(at)
                return ats

            def at_off(h, jj):
                g = (h % 4) % 2
                return g * 512 + jj * 256 + (h // 4) * 128

            def av_yn(ats, p):
                """AV matmuls + normalize (divide) for lines (2p, 2p+1).
                Returns the two normalized-yn SBUF tiles."""
                yns = []
                for jj in range(2):
                    j = (2 * p + jj) % WBLK
                    y_ps = ps_vy.tile([128, HEADS * 33], DT_F, tag="vy")
                    for h in range(HEADS):
                        half = (h % 4) // 2
                        nc.tensor.matmul(
                            y_ps[:, h * 33 : h * 33 + 33],
                            lhsT=ats[half][:, at_off(h, jj) : at_off(h, jj) + 128],
                            rhs=v_tiles[j][:, h * 33 : h * 33 + 33],
                            start=True,
                            stop=True,
                        )
                    y3 = y_ps[:].rearrange("p (a b) -> p a b", a=HEADS, b=33)
                    yn = yn_pool.tile([128, C], DT_B, tag="yn")
                    i0, i1 = broadcast_tensor_aps(y3[:, :, 0:32], y3[:, :, 32:33])
                    nc.vector.tensor_tensor(
                        yn[:].rearrange("p (a b) -> p a b", a=HEADS, b=32),
                        i0, i1, DIV,
                    )
                    yns.append(yn)
                return yns

            def tail_tr(yns, p, yt):
                """transpose + store Y^T into the block buffer."""
                for jj in range(2):
                    j = (2 * p + jj) % WBLK
                    jc = slice(j * 128, (j + 1) * 128)
                    yn = yns[jj]
                    tr = ps_ms.tile([128, 256], DT_B, tag="ms")
                    nc.tensor.transpose(tr[:, 0:128], yn[:, 0:128], id_sb[:])
                    nc.tensor.transpose(tr[:, 128:256], yn[:, 128:256], id_sb[:])
                    nc.vector.tensor_copy(
                        yt[:].rearrange("p (a l) -> p a l", a=2)[:, :, jc],
                        tr[:].rearrange("p (a l) -> p a l", a=2),
                    )

            def proj_tile(wb, yt, i, ots):
                """one out-projection psum tile (i in 0..8) + biased copy;
                fires the half-output DMA after tiles 3 and 7."""
                ct, ch = i // 4, i % 4
                if ch == 0:
                    ot = out_pool.tile([128, RBLK], DT_F, tag="ot")
                    ots[ct] = ot
                ot = ots[ct]
                ps = ps_sc.tile([128, 512], DT_F, tag="sc")
                nc.tensor.matmul(
                    ps[:],
                    lhsT=wo1[:, ct * 128 : (ct + 1) * 128],
                    rhs=yt[:, ch * 512 : (ch + 1) * 512],
                    start=True, stop=False,
                )
                nc.tensor.matmul(
                    ps[:],
                    lhsT=wo2[:, ct * 128 : (ct + 1) * 128],
                    rhs=yt[:, RBLK + ch * 512 : RBLK + (ch + 1) * 512],
                    start=False, stop=True,
                )
                # psum cols are already (w 4, h 128) == the w-major layout
                nc.scalar.activation(
                    ot[:, ch * 512 : (ch + 1) * 512],
                    ps[:],
                    AF.Identity,
                    bias=bout_sb[:, ct : ct + 1],
                )
                if ch == 3:
                    w0 = wb * WBLK
                    nc.sync.dma_start(
                        out=out_wt[ct * 128 : (ct + 1) * 128, w0 : w0 + WBLK, :],
                        in_=ot[:],
                    )

            # ================= flat slot pipeline =================
            NPAIR = NBLK * NBLK  # 64
            qk_by = {}
            yts = {}
            at_state = {}
            yn_state = {}
            ots_by = {}

            # prologue: block 0 stage-A (x-chunk arrival order) + V lines
            qk_by[0] = sa_alloc()
            for hq in range(4):
                for ft in range(4):
                    sa_chunk(0, qk_by[0], ft, hq)
            for j in range(WBLK):
                v_line(0, j)
            yts[0] = yt_pool.tile([128, 2 * RBLK], DT_B, tag="yt", name="yt0")

            for s in range(NPAIR + 2):
                if s < NPAIR:
                    wb, p = divmod(s, NBLK)
                    if p == 0 and wb > 0:
                        yts[wb] = yt_pool.tile([128, 2 * RBLK], DT_B, tag="yt", name=f"yt{wb}")
                    at_state[s] = scores_exp_eb(qk_by[wb], p)
                if s >= 1 and s - 1 < NPAIR:
                    wb1, p1 = divmod(s - 1, NBLK)
                    yn_state[s - 1] = av_yn(at_state.pop(s - 1), p1)
                if s >= 2 and s - 2 < NPAIR:
                    wb2, p2 = divmod(s - 2, NBLK)
                    tail_tr(yn_state.pop(s - 2), p2, yts[wb2])
                if s >= NPAIR:
                    continue
                # ---- phase-scheduled fillers (thin bursts) ----
                # prev block's projection: ch = p at slots 0..3 (2 tiles/slot)
                if wb >= 1 and p <= 3:
                    if p == 0:
                        ots_by[wb - 1] = {}
                    for ct in range(2):
                        proj_tile(wb - 1, yts[wb - 1], ct * 4 + p,
                                  ots_by[wb - 1])
                # last block's projection pulled into its own tail slots
                if wb == NBLK - 1:
                    if p == 4:
                        ots_by[wb] = {}
                        proj_tile(wb, yts[wb], 0, ots_by[wb])
                        proj_tile(wb, yts[wb], 4, ots_by[wb])
                    elif p == 6:
                        proj_tile(wb, yts[wb], 1, ots_by[wb])
                        proj_tile(wb, yts[wb], 5, ots_by[wb])
                # next block's stage A: slots 1..6, counts 3,3,3,3,2,2
                if wb < NBLK - 1 and 1 <= p <= 6:
                    if p == 1:
                        qk_by[wb + 1] = sa_alloc()
                    base = [0, 3, 6, 9, 12, 14][p - 1]
                    cnt = [3, 3, 3, 3, 2, 2][p - 1]
                    for k in range(base, base + cnt):
                        ft, hq = divmod(k, 4)
                        sa_chunk(wb + 1, qk_by[wb + 1], ft, hq)
                # next block's V lines: 2 per slot at p=1..7, last 2 lines
                # early in the next block (after their WAR pairs drain)
                if wb < NBLK - 1 and 1 <= p <= 7:
                    for j in (2 * (p - 1), 2 * (p - 1) + 1):
                        v_line(wb + 1, j)
                if wb >= 1 and p == 0:
                    for j in (14, 15):
                        v_line(wb, j)

            # epilogue: last block's remaining projection (ct-major so the
            # first half-output DMA fires as early as possible)
            for i in (2, 3, 6, 7):
                proj_tile(NBLK - 1, yts[NBLK - 1], i, ots_by[NBLK - 1])

    nc.compile()
    return nc


_NC = None


def _get_nc():
    global _NC
    if _NC is None:
        _NC = build_program()
    return _NC


def _prep_small(rel_bias, Wqkv, bqkv, Wout, bout):
    # bf16 blob: [w1 768 | w2 768 | wo1 256 | wo2 256 | expbt 1024 | id 128]
    w12 = Wqkv.reshape(2, 128, F)
    wo12 = Wout.reshape(2, 128, C)
    expbt_a = np.exp(rel_bias.transpose(0, 2, 1))  # [hd, m, l]
    # head order (0,4),(1,5),(2,6),(3,7): pair (hd, hd+4) shares a PE row
    # band, so the pair's scores can share one PSUM bank safely
    expbt_a = expbt_a[[0, 4, 1, 5, 2, 6, 3, 7]]
    eb = expbt_a.transpose(1, 0, 2).reshape(128, HEADS * 128)  # [m, (hd, l)]
    cb16 = np.concatenate(
        [w12[0], w12[1], wo12[0], wo12[1], eb, np.eye(128, dtype=np.float32)],
        axis=1,
    ).astype(BF16)
    # raw biases (softmax scale folded into the exp activation's scale)
    bqk_a = np.stack(
        [bqkv[0:128], bqkv[128:256], bqkv[256:384], bqkv[384:512]],
        axis=1,
    )
    bout2_a = (bout + bqkv[512:] @ Wout).reshape(2, 128).T
    cf32 = np.concatenate([bqk_a, bout2_a], axis=1).astype(np.float32)
    return {"cb16": np.ascontiguousarray(cb16), "cf32": np.ascontiguousarray(cf32)}


def _run(x, rel_bias, Wqkv, bqkv, Wout, bout, **spmd_kwargs):
    x = np.asarray(x, dtype=np.float32)
    small = _prep_small(
        np.asarray(rel_bias, np.float32),
        np.asarray(Wqkv, np.float32),
        np.asarray(bqkv, np.float32),
        np.asarray(Wout, np.float32),
        np.asarray(bout, np.float32),
    )
    nc = _get_nc()
    core_ids = list(range(8))
    in_maps = []
    for i in core_ids:
        b, t = divmod(i, T)
        m = dict(small)
        # host transpose to [C, W, H] (w-major chunks)
        m["x_wt"] = np.ascontiguousarray(
            x[b, t].transpose(0, 2, 1)
        ).astype(BF16)
        in_maps.append(m)
    res = run_bass_kernel_spmd(nc, in_maps, core_ids, **spmd_kwargs)
    out = np.empty((B, T, C, H, W), np.float32)
    for i in core_ids:
        b, t = divmod(i, T)
        out[b, t] = res.results[i]["out_wt"].transpose(0, 2, 1)
    return out, res


def kernel(x, rel_bias, Wqkv, bqkv, Wout, bout):
    out, _ = _run(x, rel_bias, Wqkv, bqkv, Wout, bout)
    return out
